# revision 3
# baseline (speedup 1.0000x reference)
"""TP-over-heads DeepseekAttention for 8 TRN2 cores, AllToAll version (v2b).

Per-core: V projection (baseline-style), Q/K projections at 512-wide
s-chunks with RoPE (q/k SBUF-resident in bf16, no DRAM spill), head-outer
attention writing bf16 outputs into per-destination AllToAll buffers (two
A2A chunks: heads 0-1 fire mid-attention and are fully hidden; heads 2-3
fire at the end and overlap o_proj block 0), then a local o_proj against
the full d-reordered bf16 Wo streamed from DRAM. No reduction collective;
host concatenates per-core [256, 4096] row slices.
"""

import numpy as np
import ml_dtypes

import concourse.bass as bass
import concourse.mybir as mybir
import concourse.tile as tile
from concourse import bacc
from concourse.bass_utils import run_bass_kernel_spmd

# problem shapes (hardcoded per contract)
S = 2048
H = 4096
NH = 32
D = 128
NC = 8
HPC = NH // NC          # 4 heads per core
DPC = HPC * D           # 512 head-dims per core
KT = H // 128           # 32 contraction tiles over hidden
SCH = 512               # s-chunk for Q/K projections
NSC = S // SCH          # 4
ST = S // 128           # 16 s-tiles
QCH = 512               # q-chunk in attention
NQC = S // QCH          # 4
NKT = S // 128          # 16 k-tiles in attention
SPC = S // NC           # 256 rows of output per core
NT = 32                 # o_proj contraction tiles: 2 blocks x 8 src x 2
OCH = 512               # o_proj output chunk width
NOC = H // OCH          # 8

f32 = mybir.dt.float32
f32r = mybir.dt.float32r
bf16 = mybir.dt.bfloat16
bf16_np = ml_dtypes.bfloat16

ROPE_THETA = 10000.0
SCALE = float(1.0 / np.sqrt(D))

_CACHE: dict = {}


def _build(with_collective=True):
    nc = bacc.Bacc("TRN2", target_bir_lowering=False, debug=False, num_devices=NC)

    # ---- I/O ----
    xt = nc.dram_tensor("xt", [KT, 128, S], bf16, kind="ExternalInput").ap()
    wq = nc.dram_tensor("wq", [KT, 128, DPC], bf16, kind="ExternalInput").ap()
    wk = nc.dram_tensor("wk", [KT, 128, DPC], bf16, kind="ExternalInput").ap()
    wv = nc.dram_tensor("wv", [KT, 128, DPC], bf16, kind="ExternalInput").ap()
    # full Wo^T, rows reordered to the A2A arrival order:
    # kt = blk*16 + src*2 + t  ->  WoT[src*512 + blk*256 + t*128 : +128, :]
    wo = nc.dram_tensor("wo", [NT, 128, H], bf16, kind="ExternalInput").ap()
    cost = nc.dram_tensor("cost", [128, S], f32, kind="ExternalInput").ap()
    sint = nc.dram_tensor("sint", [128, S], f32, kind="ExternalInput").ap()
    rmat = nc.dram_tensor("rmat", [128, 128], f32r, kind="ExternalInput").ap()
    ones_col = nc.dram_tensor("ones_col", [128, 1], f32r, kind="ExternalInput").ap()
    ones_row = nc.dram_tensor("ones_row", [1, 128], f32r, kind="ExternalInput").ap()
    out_ext = nc.dram_tensor("out", [SPC, H], f32, kind="ExternalOutput").ap()

    with tile.TileContext(nc) as tc:
        with (
            tc.tile_pool(name="dram", bufs=1, space="DRAM") as dram_pool,
            tc.tile_pool(name="vstore", bufs=1) as v_store,
            tc.tile_pool(name="qkstore", bufs=1) as qk_store,
        ):
            a2a_in = [
                dram_pool.tile([NC, 2 * D, SPC], bf16, name=f"a2a_in{b}",
                               tag=f"a2a_in{b}")
                for b in range(2)
            ]
            a2a_out = [
                dram_pool.tile([NC, 2 * D, SPC], bf16,
                               name=f"a2a_out{b}", tag=f"a2a_out{b}")
                for b in range(2)
            ]
            q_sb = qk_store.tile([128, HPC, S], bf16, tag="qs")
            k_sb = qk_store.tile([128, HPC, S], bf16, tag="ks")
            v_tiles = []

            with (
                tc.tile_pool(name="wqk", bufs=1) as wqk_pool,
                tc.tile_pool(name="xt1", bufs=1) as xt1_pool,
            ):
                wq_sb = wqk_pool.tile([128, KT, DPC], bf16, tag="wq")
                wk_sb = wqk_pool.tile([128, KT, DPC], bf16, tag="wk")
                # prefetch the first Q/K-phase x chunk on the idle gpsimd
                # queue so the QK matmuls start right after phase V
                x0_sb = xt1_pool.tile([128, KT, SCH], bf16, tag="x",
                                      name="x0")
                for g in range(4):
                    nc.gpsimd.dma_start(
                        x0_sb[:, 8 * g:8 * (g + 1), :],
                        xt.rearrange("k p s -> p k s")[:, 8 * g:8 * (g + 1),
                                                       0:SCH],
                    )

                # ====== Phase V: V projection (natural [s, d] layout) ========
                with (
                    tc.tile_pool(name="wv", bufs=1) as wv_pool,
                    tc.tile_pool(name="xt2", bufs=3) as xt2_pool,
                    tc.tile_pool(name="psB", bufs=2, space="PSUM") as psB,
                ):
                    wv_sb = wv_pool.tile([128, KT, DPC], bf16, tag="wv")
                    # first wv group goes first so V matmuls start ASAP;
                    # the first x s-tile is interleaved right behind it
                    nc.sync.dma_start(
                        wv_sb[:, 0:4, :],
                        wv.rearrange("k p n -> p k n")[:, 0:4, :],
                    )
                    # prefetch Wq/Wk on the scalar-engine HWDGE queue
                    for g in range(4):
                        nc.scalar.dma_start(
                            wq_sb[:, :, 128 * g:128 * (g + 1)],
                            wq.rearrange("k p n -> p k n")[:, :, 128 * g:128 * (g + 1)],
                        )
                        nc.scalar.dma_start(
                            wk_sb[:, :, 128 * g:128 * (g + 1)],
                            wk.rearrange("k p n -> p k n")[:, :, 128 * g:128 * (g + 1)],
                        )
                    for st in range(ST):
                        x_sb = xt2_pool.tile([128, KT, 128], bf16, tag="x2")
                        nsplit = 4 if st == 0 else 1
                        for g in range(nsplit):
                            kspan = KT // nsplit
                            nc.sync.dma_start(
                                x_sb[:, kspan * g:kspan * (g + 1), :],
                                xt.rearrange("k p s -> p k s")[
                                    :, kspan * g:kspan * (g + 1),
                                    st * 128:(st + 1) * 128],
                            )
                        if st == 0:
                            # remaining wv groups, behind the first x tile
                            for g in range(1, 8):
                                nc.sync.dma_start(
                                    wv_sb[:, 4 * g:4 * (g + 1), :],
                                    wv.rearrange("k p n -> p k n")[
                                        :, 4 * g:4 * (g + 1), :],
                                )
                        ps = psB.tile([128, DPC], f32, tag="vp")
                        for kt in range(KT):
                            nc.tensor.matmul(
                                ps[:], x_sb[:, kt, :], wv_sb[:, kt, :],
                                start=(kt == 0), stop=(kt == KT - 1),
                            )
                        v_t = v_store.tile([128, DPC], bf16, tag=f"v{st}",
                                           name=f"v{st}")
                        nc.scalar.copy(v_t[:], ps[:])
                        v_tiles.append(v_t)

                # ====== Phase QK: Q/K projections + RoPE -> SBUF bf16 ========
                with (
                    tc.tile_pool(name="ropec", bufs=2) as rope_pool,
                    tc.tile_pool(name="rmp", bufs=1) as rm_pool,
                    tc.tile_pool(name="qktmp", bufs=2) as qktmp_pool,
                    tc.tile_pool(name="psA", bufs=2, space="PSUM") as psA,
                ):
                    rm_sb = rm_pool.tile([128, 128], f32r, tag="rm")
                    nc.sync.dma_start(rm_sb[:], rmat[:])
                    for sc in range(NSC):
                        s0 = sc * SCH
                        if sc == 0:
                            x_sb = x0_sb
                        else:
                            x_sb = xt1_pool.tile([128, KT, SCH], bf16, tag="x")
                            nc.sync.dma_start(
                                x_sb[:],
                                xt.rearrange("k p s -> p k s")[:, :,
                                                               s0:s0 + SCH],
                            )
                        cos_sb = rope_pool.tile([128, SCH], f32, tag="cos")
                        sin_sb = rope_pool.tile([128, SCH], f32, tag="sin")
                        nc.scalar.dma_start(cos_sb[:], cost[:, s0:s0 + SCH])
                        nc.scalar.dma_start(sin_sb[:], sint[:, s0:s0 + SCH])
                        for pi, w_sb, dst in ((0, wq_sb, q_sb), (1, wk_sb, k_sb)):
                            for h in range(HPC):
                                ps = psA.tile([128, SCH], f32, tag="proj")
                                for kt in range(KT):
                                    nc.tensor.matmul(
                                        ps[:],
                                        w_sb[:, kt, h * 128:(h + 1) * 128],
                                        x_sb[:, kt, :],
                                        start=(kt == 0),
                                        stop=(kt == KT - 1),
                                    )
                                raw = qktmp_pool.tile([128, SCH], f32r, tag="raw")
                                nc.scalar.copy(raw[:], ps[:])
                                psr = psA.tile([128, SCH], f32, tag="rot")
                                nc.tensor.matmul(psr[:], rm_sb[:], raw[:],
                                                 start=True, stop=True)
                                t1 = qktmp_pool.tile([128, SCH], f32, tag="t1")
                                nc.vector.tensor_mul(t1[:], raw[:], cos_sb[:])
                                t2 = qktmp_pool.tile([128, SCH], f32, tag="t2")
                                nc.vector.tensor_mul(t2[:], psr[:], sin_sb[:])
                                nc.vector.tensor_add(
                                    dst[:, h, s0:s0 + SCH], t1[:], t2[:])

            # ====== Phase attn: head-outer + chunked A2A ====================
            with (
                tc.tile_pool(name="wo", bufs=2) as wo_pool,
                tc.tile_pool(name="attst", bufs=1) as att_pool,
                tc.tile_pool(name="accp", bufs=1) as acc_pool,
                tc.tile_pool(name="drain", bufs=4) as drain_pool,
                tc.tile_pool(name="psC", bufs=1, space="PSUM") as psC,
                tc.tile_pool(name="pt", bufs=10) as pt_pool,
                tc.tile_pool(name="tmp", bufs=1) as tmp_pool,
                tc.tile_pool(name="attnmisc", bufs=3) as misc_pool,
                tc.tile_pool(name="otp", bufs=3) as ot_pool,
            ):
                att_src = a2a_out if with_collective else a2a_in
                att_sbs = [
                    att_pool.tile([128, NT // 2, SPC], bf16, tag=f"att{b}",
                                  name=f"att{b}")
                    for b in range(2)
                ]
                oc_sb = misc_pool.tile([128, 1], f32r, tag="ones_c", bufs=1)
                or_sb = misc_pool.tile([1, 128], f32r, tag="ones_r", bufs=1)
                nc.sync.dma_start(oc_sb[:], ones_col[:])
                nc.sync.dma_start(or_sb[:], ones_row[:])

                for h in range(HPC):
                    blk, hb = h // 2, h % 2
                    for qc in range(NQC):
                        q0 = qc * QCH
                        # scores^T + exp, interleaved with attn@V accumulation
                        ps_o = psC.tile([128, QCH], f32, tag="vmm", bufs=2)
                        pts = []
                        for kt in range(NKT):
                            ps_s = psC.tile([128, QCH], f32, tag="scores",
                                            bufs=3)
                            nc.tensor.matmul(
                                ps_s[:],
                                k_sb[:, h, kt * 128:(kt + 1) * 128],
                                q_sb[:, h, q0:q0 + QCH],
                                start=True, stop=True,
                            )
                            pt = pt_pool.tile([128, QCH], bf16, tag="pt")
                            nc.scalar.activation(
                                pt[:], ps_s[:],
                                mybir.ActivationFunctionType.Exp, scale=SCALE,
                            )
                            pts.append(pt)
                            if kt >= 2:
                                kv = kt - 2
                                nc.tensor.matmul(
                                    ps_o[:],
                                    v_tiles[kv][:, h * 128:(h + 1) * 128],
                                    pts[kv][:],
                                    start=(kv == 0), stop=False,
                                )
                        for kv in (NKT - 2, NKT - 1):
                            nc.tensor.matmul(
                                ps_o[:],
                                v_tiles[kv][:, h * 128:(h + 1) * 128],
                                pts[kv][:],
                                start=False, stop=(kv == NKT - 1),
                            )

                        # denominator: batched tree sum of the 16 P^T tiles
                        tmp = tmp_pool.tile([128, 8, QCH], f32, tag="tr")
                        for i in range(8):
                            nc.vector.tensor_add(tmp[:, i, :],
                                                 pts[2 * i][:], pts[2 * i + 1][:])
                        nc.vector.tensor_add(tmp[:, 0:4, :],
                                             tmp[:, 0:4, :], tmp[:, 4:8, :])
                        nc.vector.tensor_add(tmp[:, 0:2, :],
                                             tmp[:, 0:2, :], tmp[:, 2:4, :])
                        t_sum = misc_pool.tile([128, QCH], f32r, tag="tsum",
                                               bufs=2)
                        nc.vector.tensor_add(t_sum[:], tmp[:, 0, :], tmp[:, 1, :])

                        # cross-partition sum -> broadcast -> reciprocal
                        ps_sum = psC.tile([1, QCH], f32, tag="sumbc", bufs=1)
                        nc.tensor.matmul(ps_sum[:], oc_sb[:], t_sum[:],
                                         start=True, stop=True)
                        sum_sb = misc_pool.tile([1, QCH], f32r, tag="sum_sb")
                        nc.vector.tensor_copy(sum_sb[:], ps_sum[:])
                        ps_bc = psC.tile([128, QCH], f32, tag="sumbc", bufs=1)
                        nc.tensor.matmul(ps_bc[:], or_sb[:], sum_sb[:],
                                         start=True, stop=True)
                        recip_sb = misc_pool.tile([128, QCH], f32, tag="recip")
                        nc.vector.reciprocal(recip_sb[:], ps_bc[:])

                        ot_t = ot_pool.tile([128, QCH], bf16, tag="ot")
                        nc.vector.tensor_mul(ot_t[:], ps_o[:], recip_sb[:])
                        # scatter this q-chunk's halves to their dest ranks
                        for half in range(2):
                            dest = 2 * qc + half
                            nc.sync.dma_start(
                                a2a_in[blk][dest,
                                            hb * 128:(hb + 1) * 128, :],
                                ot_t[:, half * SPC:(half + 1) * SPC],
                            )

                    if with_collective and hb == 1:
                        nc.gpsimd.collective_compute(
                            "AllToAll",
                            mybir.AluOpType.bypass,
                            replica_groups=[list(range(NC))],
                            ins=[a2a_in[blk][:].opt()],
                            outs=[a2a_out[blk][:].opt()],
                        )
                    if hb == 1:
                        # stage this block's A2A result into SBUF right away
                        # (gpsimd queue; waits on the collective, overlaps
                        # the remaining attention / o_proj compute)
                        nc.gpsimd.dma_start(
                            att_sbs[blk][:],
                            att_src[blk].rearrange("r (t p) s -> p (r t) s",
                                                   p=128),
                        )

                # ====== Phase o_proj: two blocks, blk0 overlaps A2A#2 =======
                acc_sb = acc_pool.tile([128, SPC // 128, H], f32, tag="acc")
                for blk in range(2):
                    att_sb = att_sbs[blk]
                    for nci in range(NOC):
                        n0 = nci * OCH
                        wo_sb = wo_pool.tile([128, NT // 2, OCH], bf16, tag="wo",
                                             bufs=3)
                        # split the chunk across two DMA queues: the wo
                        # stream is the o_proj bottleneck on one queue
                        nc.scalar.dma_start(
                            wo_sb[:, 0:NT // 4, :],
                            wo.rearrange("k p n -> p k n")[
                                :, blk * (NT // 2):blk * (NT // 2) + NT // 4,
                                n0:n0 + OCH],
                        )
                        nc.sync.dma_start(
                            wo_sb[:, NT // 4:NT // 2, :],
                            wo.rearrange("k p n -> p k n")[
                                :, blk * (NT // 2) + NT // 4:
                                (blk + 1) * (NT // 2),
                                n0:n0 + OCH],
                        )
                        for st in range(SPC // 128):
                            ps = psC.tile([128, OCH], f32, tag="op", bufs=2)
                            for ckt in range(NT // 2):
                                nc.tensor.matmul(
                                    ps[:],
                                    att_sb[:, ckt, st * 128:(st + 1) * 128],
                                    wo_sb[:, ckt, :],
                                    start=(ckt == 0),
                                    stop=(ckt == NT // 2 - 1),
                                )
                            if blk == 0:
                                nc.vector.tensor_copy(
                                    acc_sb[:, st, n0:n0 + OCH], ps[:])
                            else:
                                dr = drain_pool.tile([128, OCH], f32, tag="dr")
                                nc.vector.tensor_add(
                                    dr[:], ps[:], acc_sb[:, st, n0:n0 + OCH])
                                nc.gpsimd.dma_start(
                                    out_ext[st * 128:(st + 1) * 128,
                                            n0:n0 + OCH],
                                    dr[:],
                                )

    nc.compile()
    return nc


def _host_prep(positions, hidden_states, Wq, Wk, Wv, Wo):
    X = np.asarray(hidden_states, dtype=np.float32).reshape(S, H)
    XT = np.ascontiguousarray(X.T).astype(bf16_np).reshape(KT, 128, S)

    pos = np.asarray(positions).astype(np.float32)
    inv_freq = (1.0 / (ROPE_THETA ** (np.arange(0, D, 2, dtype=np.float32) / D)))
    freqs = pos[:, None] * inv_freq[None, :]
    emb = np.concatenate([freqs, freqs], axis=-1)        # [S, D]
    cosT = np.ascontiguousarray(np.cos(emb).astype(np.float32).T)  # [128, S]
    sinT = np.ascontiguousarray(np.sin(emb).astype(np.float32).T)

    rm = np.zeros((128, 128), np.float32)
    idx = np.arange(64)
    rm[64 + idx, idx] = -1.0   # out[0:64]  = -in[64:128]
    rm[idx, 64 + idx] = 1.0    # out[64:128] = in[0:64]

    Wq = np.asarray(Wq, dtype=np.float32)
    Wk = np.asarray(Wk, dtype=np.float32)
    Wv = np.asarray(Wv, dtype=np.float32)
    Wo = np.asarray(Wo, dtype=np.float32)

    # WoT rows in A2A arrival order: kt = blk*16 + src*2 + t covers
    # global d = src*512 + blk*256 + t*128 + (0:128)
    WoT = np.ascontiguousarray(Wo.T).astype(bf16_np)      # [d, out]
    wo_ord = WoT.reshape(NC, 2, 2, 128, H).transpose(1, 0, 2, 3, 4)
    wo_ord = np.ascontiguousarray(wo_ord).reshape(NT, 128, H)

    in_maps = []
    for c in range(NC):
        sl = slice(DPC * c, DPC * (c + 1))
        wq_c = np.ascontiguousarray(Wq[sl, :].T).astype(bf16_np).reshape(KT, 128, DPC)
        wk_c = np.ascontiguousarray(Wk[sl, :].T).astype(bf16_np).reshape(KT, 128, DPC)
        wv_c = np.ascontiguousarray(Wv[sl, :].T).astype(bf16_np).reshape(KT, 128, DPC)
        in_maps.append({
            "xt": XT, "wq": wq_c, "wk": wk_c, "wv": wv_c, "wo": wo_ord,
            "cost": cosT, "sint": sinT, "rmat": rm,
            "ones_col": np.ones((128, 1), np.float32),
            "ones_row": np.ones((1, 128), np.float32),
        })
    return in_maps


def _assemble(results):
    """Core c holds global output rows [256c, 256(c+1))."""
    out = np.concatenate([results[c]["out"] for c in range(NC)], axis=0)
    return out.reshape(1, S, H)


def kernel(positions, hidden_states, Wq, Wk, Wv, Wo):
    if "nc" not in _CACHE:
        _CACHE["nc"] = _build()
    nc = _CACHE["nc"]
    in_maps = _host_prep(positions, hidden_states, Wq, Wk, Wv, Wo)
    res = run_bass_kernel_spmd(nc, in_maps, list(range(NC)))
    return _assemble(res.results).astype(np.float32)


# revision 4
# speedup vs baseline: 1.2007x; 1.2007x over previous
"""TP-over-heads DeepseekAttention for 8 TRN2 cores, AllToAll version (v2b).

Per-core: V projection (baseline-style), Q/K projections at 512-wide
s-chunks with RoPE (q/k SBUF-resident in bf16, no DRAM spill), head-outer
attention writing bf16 outputs into per-destination AllToAll buffers (two
A2A chunks: heads 0-1 fire mid-attention and are fully hidden; heads 2-3
fire at the end and overlap o_proj block 0), then a local o_proj against
the full d-reordered bf16 Wo streamed from DRAM. No reduction collective;
host concatenates per-core [256, 4096] row slices.
"""

import numpy as np
import ml_dtypes

import concourse.bass as bass
import concourse.mybir as mybir
import concourse.tile as tile
from concourse import bacc
from concourse.bass_utils import run_bass_kernel_spmd

# problem shapes (hardcoded per contract)
S = 2048
H = 4096
NH = 32
D = 128
NC = 8
HPC = NH // NC          # 4 heads per core
DPC = HPC * D           # 512 head-dims per core
KT = H // 128           # 32 contraction tiles over hidden
SCH = 512               # s-chunk for Q/K projections
NSC = S // SCH          # 4
ST = S // 128           # 16 s-tiles
QCH = 512               # q-chunk in attention
NQC = S // QCH          # 4
NKT = S // 128          # 16 k-tiles in attention
SPC = S // NC           # 256 rows of output per core
NT = 32                 # o_proj contraction tiles: 2 blocks x 8 src x 2
OCH = 512               # o_proj output chunk width
NOC = H // OCH          # 8

f32 = mybir.dt.float32
f32r = mybir.dt.float32r
bf16 = mybir.dt.bfloat16
bf16_np = ml_dtypes.bfloat16

ROPE_THETA = 10000.0
SCALE = float(1.0 / np.sqrt(D))

_CACHE: dict = {}


def _build(with_collective=True):
    nc = bacc.Bacc("TRN2", target_bir_lowering=False, debug=False, num_devices=NC)

    # ---- I/O ----
    xt = nc.dram_tensor("xt", [KT, 128, S], bf16, kind="ExternalInput").ap()
    wq = nc.dram_tensor("wq", [KT, 128, DPC], bf16, kind="ExternalInput").ap()
    wk = nc.dram_tensor("wk", [KT, 128, DPC], bf16, kind="ExternalInput").ap()
    wv = nc.dram_tensor("wv", [KT, 128, DPC], bf16, kind="ExternalInput").ap()
    # full Wo^T, rows reordered to the A2A arrival order:
    # kt = blk*16 + src*2 + t  ->  WoT[src*512 + blk*256 + t*128 : +128, :]
    wo = nc.dram_tensor("wo", [NT, 128, H], bf16, kind="ExternalInput").ap()
    cost = nc.dram_tensor("cost", [128, S], f32, kind="ExternalInput").ap()
    sint = nc.dram_tensor("sint", [128, S], f32, kind="ExternalInput").ap()
    rmat = nc.dram_tensor("rmat", [128, 128], f32r, kind="ExternalInput").ap()
    ones_col = nc.dram_tensor("ones_col", [128, 1], f32r, kind="ExternalInput").ap()
    ones_row = nc.dram_tensor("ones_row", [1, 128], f32r, kind="ExternalInput").ap()
    out_ext = nc.dram_tensor("out", [SPC, H], f32, kind="ExternalOutput").ap()

    with tile.TileContext(nc) as tc:
        with (
            tc.tile_pool(name="dram", bufs=1, space="DRAM") as dram_pool,
            tc.tile_pool(name="vstore", bufs=1) as v_store,
            tc.tile_pool(name="qkstore", bufs=1) as qk_store,
        ):
            a2a_in = [
                dram_pool.tile([NC, 2 * D, SPC], bf16, name=f"a2a_in{b}",
                               tag=f"a2a_in{b}")
                for b in range(2)
            ]
            a2a_out = [
                dram_pool.tile([NC, 2 * D, SPC], bf16,
                               name=f"a2a_out{b}", tag=f"a2a_out{b}")
                for b in range(2)
            ]
            q_sb = qk_store.tile([128, HPC, S], bf16, tag="qs")
            k_sb = qk_store.tile([128, HPC, S], bf16, tag="ks")
            v_tiles = []

            with (
                tc.tile_pool(name="wqk", bufs=1) as wqk_pool,
                tc.tile_pool(name="xt1", bufs=1) as xt1_pool,
            ):
                wq_sb = wqk_pool.tile([128, KT, DPC], bf16, tag="wq")
                wk_sb = wqk_pool.tile([128, KT, DPC], bf16, tag="wk")
                # prefetch the first Q/K-phase x chunk on the idle gpsimd
                # queue so the QK matmuls start right after phase V
                x0_sb = xt1_pool.tile([128, KT, SCH], bf16, tag="x",
                                      name="x0")
                for g in range(4):
                    nc.gpsimd.dma_start(
                        x0_sb[:, 8 * g:8 * (g + 1), :],
                        xt.rearrange("k p s -> p k s")[:, 8 * g:8 * (g + 1),
                                                       0:SCH],
                    )

                # ====== Phase V: V projection, kt-group outer ================
                # The first s-tile's PSUM chain would otherwise wait for the
                # full 4.2MB wv to arrive. Sweeping kt-groups in the outer
                # loop with 8 concurrent PSUM banks (one per s-tile) lets the
                # matmuls trail the wv DMA stream group by group.
                NG = 8          # kt groups of 4
                GK = KT // NG
                with (
                    tc.tile_pool(name="wv", bufs=1) as wv_pool,
                    tc.tile_pool(name="xg", bufs=2) as xg_pool,
                    tc.tile_pool(name="psB", bufs=1, space="PSUM") as psB,
                ):
                    wv_sb = wv_pool.tile([128, KT, DPC], bf16, tag="wv")
                    for g in range(NG):
                        nc.sync.dma_start(
                            wv_sb[:, GK * g:GK * (g + 1), :],
                            wv.rearrange("k p n -> p k n")[:, GK * g:GK * (g + 1), :],
                        )
                    # prefetch Wq/Wk on the scalar-engine HWDGE queue
                    for g in range(4):
                        nc.scalar.dma_start(
                            wq_sb[:, :, 128 * g:128 * (g + 1)],
                            wq.rearrange("k p n -> p k n")[:, :, 128 * g:128 * (g + 1)],
                        )
                        nc.scalar.dma_start(
                            wk_sb[:, :, 128 * g:128 * (g + 1)],
                            wk.rearrange("k p n -> p k n")[:, :, 128 * g:128 * (g + 1)],
                        )
                    for half in range(2):
                        s0 = half * (S // 2)
                        ps_tiles = [
                            psB.tile([128, DPC], f32, tag=f"vp{i}",
                                     name=f"vp{half}_{i}")
                            for i in range(8)
                        ]
                        for g in range(NG):
                            xg = xg_pool.tile([128, GK, S // 2], bf16,
                                              tag="xg")
                            nc.sync.dma_start(
                                xg[:],
                                xt.rearrange("k p s -> p k s")[
                                    :, GK * g:GK * (g + 1), s0:s0 + S // 2],
                            )
                            for i in range(8):
                                for kl in range(GK):
                                    nc.tensor.matmul(
                                        ps_tiles[i][:],
                                        xg[:, kl, i * 128:(i + 1) * 128],
                                        wv_sb[:, GK * g + kl, :],
                                        start=(g == 0 and kl == 0),
                                        stop=(g == NG - 1 and kl == GK - 1),
                                    )
                        for i in range(8):
                            st = half * 8 + i
                            v_t = v_store.tile([128, DPC], bf16, tag=f"v{st}",
                                               name=f"v{st}")
                            nc.scalar.copy(v_t[:], ps_tiles[i][:])
                            v_tiles.append(v_t)

                # ====== Phase QK: Q/K projections + RoPE -> SBUF bf16 ========
                with (
                    tc.tile_pool(name="ropec", bufs=2) as rope_pool,
                    tc.tile_pool(name="rmp", bufs=1) as rm_pool,
                    tc.tile_pool(name="qktmp", bufs=2) as qktmp_pool,
                    tc.tile_pool(name="psA", bufs=2, space="PSUM") as psA,
                ):
                    rm_sb = rm_pool.tile([128, 128], f32r, tag="rm")
                    nc.sync.dma_start(rm_sb[:], rmat[:])
                    for sc in range(NSC):
                        s0 = sc * SCH
                        if sc == 0:
                            x_sb = x0_sb
                        else:
                            x_sb = xt1_pool.tile([128, KT, SCH], bf16, tag="x")
                            nc.sync.dma_start(
                                x_sb[:],
                                xt.rearrange("k p s -> p k s")[:, :,
                                                               s0:s0 + SCH],
                            )
                        cos_sb = rope_pool.tile([128, SCH], f32, tag="cos")
                        sin_sb = rope_pool.tile([128, SCH], f32, tag="sin")
                        nc.scalar.dma_start(cos_sb[:], cost[:, s0:s0 + SCH])
                        nc.scalar.dma_start(sin_sb[:], sint[:, s0:s0 + SCH])
                        for pi, w_sb, dst in ((0, wq_sb, q_sb), (1, wk_sb, k_sb)):
                            for h in range(HPC):
                                ps = psA.tile([128, SCH], f32, tag="proj")
                                for kt in range(KT):
                                    nc.tensor.matmul(
                                        ps[:],
                                        w_sb[:, kt, h * 128:(h + 1) * 128],
                                        x_sb[:, kt, :],
                                        start=(kt == 0),
                                        stop=(kt == KT - 1),
                                    )
                                raw = qktmp_pool.tile([128, SCH], f32r, tag="raw")
                                nc.scalar.copy(raw[:], ps[:])
                                psr = psA.tile([128, SCH], f32, tag="rot")
                                nc.tensor.matmul(psr[:], rm_sb[:], raw[:],
                                                 start=True, stop=True)
                                t1 = qktmp_pool.tile([128, SCH], f32, tag="t1")
                                nc.vector.tensor_mul(t1[:], raw[:], cos_sb[:])
                                t2 = qktmp_pool.tile([128, SCH], f32, tag="t2")
                                nc.vector.tensor_mul(t2[:], psr[:], sin_sb[:])
                                nc.vector.tensor_add(
                                    dst[:, h, s0:s0 + SCH], t1[:], t2[:])

            # ====== Phase attn: head-outer + chunked A2A ====================
            with (
                tc.tile_pool(name="wo", bufs=2) as wo_pool,
                tc.tile_pool(name="attst", bufs=1) as att_pool,
                tc.tile_pool(name="accp", bufs=1) as acc_pool,
                tc.tile_pool(name="drain", bufs=4) as drain_pool,
                tc.tile_pool(name="psC", bufs=1, space="PSUM") as psC,
                tc.tile_pool(name="pt", bufs=10) as pt_pool,
                tc.tile_pool(name="tmp", bufs=1) as tmp_pool,
                tc.tile_pool(name="attnmisc", bufs=3) as misc_pool,
                tc.tile_pool(name="otp", bufs=3) as ot_pool,
            ):
                att_src = a2a_out if with_collective else a2a_in
                att_sbs = [
                    att_pool.tile([128, NT // 2, SPC], bf16, tag=f"att{b}",
                                  name=f"att{b}")
                    for b in range(2)
                ]
                oc_sb = misc_pool.tile([128, 1], f32r, tag="ones_c", bufs=1)
                or_sb = misc_pool.tile([1, 128], f32r, tag="ones_r", bufs=1)
                nc.sync.dma_start(oc_sb[:], ones_col[:])
                nc.sync.dma_start(or_sb[:], ones_row[:])

                for h in range(HPC):
                    blk, hb = h // 2, h % 2
                    for qc in range(NQC):
                        q0 = qc * QCH
                        # scores^T + exp, interleaved with attn@V accumulation
                        ps_o = psC.tile([128, QCH], f32, tag="vmm", bufs=2)
                        pts = []
                        for kt in range(NKT):
                            ps_s = psC.tile([128, QCH], f32, tag="scores",
                                            bufs=3)
                            nc.tensor.matmul(
                                ps_s[:],
                                k_sb[:, h, kt * 128:(kt + 1) * 128],
                                q_sb[:, h, q0:q0 + QCH],
                                start=True, stop=True,
                            )
                            pt = pt_pool.tile([128, QCH], bf16, tag="pt")
                            nc.scalar.activation(
                                pt[:], ps_s[:],
                                mybir.ActivationFunctionType.Exp, scale=SCALE,
                            )
                            pts.append(pt)
                            if kt >= 2:
                                kv = kt - 2
                                nc.tensor.matmul(
                                    ps_o[:],
                                    v_tiles[kv][:, h * 128:(h + 1) * 128],
                                    pts[kv][:],
                                    start=(kv == 0), stop=False,
                                )
                        for kv in (NKT - 2, NKT - 1):
                            nc.tensor.matmul(
                                ps_o[:],
                                v_tiles[kv][:, h * 128:(h + 1) * 128],
                                pts[kv][:],
                                start=False, stop=(kv == NKT - 1),
                            )

                        # denominator: batched tree sum of the 16 P^T tiles
                        tmp = tmp_pool.tile([128, 8, QCH], f32, tag="tr")
                        for i in range(8):
                            nc.vector.tensor_add(tmp[:, i, :],
                                                 pts[2 * i][:], pts[2 * i + 1][:])
                        nc.vector.tensor_add(tmp[:, 0:4, :],
                                             tmp[:, 0:4, :], tmp[:, 4:8, :])
                        nc.vector.tensor_add(tmp[:, 0:2, :],
                                             tmp[:, 0:2, :], tmp[:, 2:4, :])
                        t_sum = misc_pool.tile([128, QCH], f32r, tag="tsum",
                                               bufs=2)
                        nc.vector.tensor_add(t_sum[:], tmp[:, 0, :], tmp[:, 1, :])

                        # cross-partition sum -> broadcast -> reciprocal
                        ps_sum = psC.tile([1, QCH], f32, tag="sumbc", bufs=1)
                        nc.tensor.matmul(ps_sum[:], oc_sb[:], t_sum[:],
                                         start=True, stop=True)
                        sum_sb = misc_pool.tile([1, QCH], f32r, tag="sum_sb")
                        nc.vector.tensor_copy(sum_sb[:], ps_sum[:])
                        ps_bc = psC.tile([128, QCH], f32, tag="sumbc", bufs=1)
                        nc.tensor.matmul(ps_bc[:], or_sb[:], sum_sb[:],
                                         start=True, stop=True)
                        recip_sb = misc_pool.tile([128, QCH], f32, tag="recip")
                        nc.vector.reciprocal(recip_sb[:], ps_bc[:])

                        ot_t = ot_pool.tile([128, QCH], bf16, tag="ot")
                        nc.vector.tensor_mul(ot_t[:], ps_o[:], recip_sb[:])
                        # scatter this q-chunk's halves to their dest ranks
                        for half in range(2):
                            dest = 2 * qc + half
                            nc.sync.dma_start(
                                a2a_in[blk][dest,
                                            hb * 128:(hb + 1) * 128, :],
                                ot_t[:, half * SPC:(half + 1) * SPC],
                            )

                    if with_collective and hb == 1:
                        nc.gpsimd.collective_compute(
                            "AllToAll",
                            mybir.AluOpType.bypass,
                            replica_groups=[list(range(NC))],
                            ins=[a2a_in[blk][:].opt()],
                            outs=[a2a_out[blk][:].opt()],
                        )
                    if hb == 1:
                        # stage this block's A2A result into SBUF right away
                        # (gpsimd queue; waits on the collective, overlaps
                        # the remaining attention / o_proj compute)
                        nc.gpsimd.dma_start(
                            att_sbs[blk][:],
                            att_src[blk].rearrange("r (t p) s -> p (r t) s",
                                                   p=128),
                        )

                # ====== Phase o_proj: two blocks, blk0 overlaps A2A#2 =======
                acc_sb = acc_pool.tile([128, SPC // 128, H], f32, tag="acc")
                for blk in range(2):
                    att_sb = att_sbs[blk]
                    for nci in range(NOC):
                        n0 = nci * OCH
                        wo_sb = wo_pool.tile([128, NT // 2, OCH], bf16, tag="wo",
                                             bufs=3)
                        # split the chunk across two DMA queues: the wo
                        # stream is the o_proj bottleneck on one queue
                        nc.scalar.dma_start(
                            wo_sb[:, 0:NT // 4, :],
                            wo.rearrange("k p n -> p k n")[
                                :, blk * (NT // 2):blk * (NT // 2) + NT // 4,
                                n0:n0 + OCH],
                        )
                        nc.sync.dma_start(
                            wo_sb[:, NT // 4:NT // 2, :],
                            wo.rearrange("k p n -> p k n")[
                                :, blk * (NT // 2) + NT // 4:
                                (blk + 1) * (NT // 2),
                                n0:n0 + OCH],
                        )
                        for st in range(SPC // 128):
                            ps = psC.tile([128, OCH], f32, tag="op", bufs=2)
                            for ckt in range(NT // 2):
                                nc.tensor.matmul(
                                    ps[:],
                                    att_sb[:, ckt, st * 128:(st + 1) * 128],
                                    wo_sb[:, ckt, :],
                                    start=(ckt == 0),
                                    stop=(ckt == NT // 2 - 1),
                                )
                            if blk == 0:
                                nc.vector.tensor_copy(
                                    acc_sb[:, st, n0:n0 + OCH], ps[:])
                            else:
                                dr = drain_pool.tile([128, OCH], f32, tag="dr")
                                nc.vector.tensor_add(
                                    dr[:], ps[:], acc_sb[:, st, n0:n0 + OCH])
                                nc.gpsimd.dma_start(
                                    out_ext[st * 128:(st + 1) * 128,
                                            n0:n0 + OCH],
                                    dr[:],
                                )

    nc.compile()
    return nc


def _host_prep(positions, hidden_states, Wq, Wk, Wv, Wo):
    X = np.asarray(hidden_states, dtype=np.float32).reshape(S, H)
    XT = np.ascontiguousarray(X.T).astype(bf16_np).reshape(KT, 128, S)

    pos = np.asarray(positions).astype(np.float32)
    inv_freq = (1.0 / (ROPE_THETA ** (np.arange(0, D, 2, dtype=np.float32) / D)))
    freqs = pos[:, None] * inv_freq[None, :]
    emb = np.concatenate([freqs, freqs], axis=-1)        # [S, D]
    cosT = np.ascontiguousarray(np.cos(emb).astype(np.float32).T)  # [128, S]
    sinT = np.ascontiguousarray(np.sin(emb).astype(np.float32).T)

    rm = np.zeros((128, 128), np.float32)
    idx = np.arange(64)
    rm[64 + idx, idx] = -1.0   # out[0:64]  = -in[64:128]
    rm[idx, 64 + idx] = 1.0    # out[64:128] = in[0:64]

    Wq = np.asarray(Wq, dtype=np.float32)
    Wk = np.asarray(Wk, dtype=np.float32)
    Wv = np.asarray(Wv, dtype=np.float32)
    Wo = np.asarray(Wo, dtype=np.float32)

    # WoT rows in A2A arrival order: kt = blk*16 + src*2 + t covers
    # global d = src*512 + blk*256 + t*128 + (0:128)
    WoT = np.ascontiguousarray(Wo.T).astype(bf16_np)      # [d, out]
    wo_ord = WoT.reshape(NC, 2, 2, 128, H).transpose(1, 0, 2, 3, 4)
    wo_ord = np.ascontiguousarray(wo_ord).reshape(NT, 128, H)

    in_maps = []
    for c in range(NC):
        sl = slice(DPC * c, DPC * (c + 1))
        wq_c = np.ascontiguousarray(Wq[sl, :].T).astype(bf16_np).reshape(KT, 128, DPC)
        wk_c = np.ascontiguousarray(Wk[sl, :].T).astype(bf16_np).reshape(KT, 128, DPC)
        wv_c = np.ascontiguousarray(Wv[sl, :].T).astype(bf16_np).reshape(KT, 128, DPC)
        in_maps.append({
            "xt": XT, "wq": wq_c, "wk": wk_c, "wv": wv_c, "wo": wo_ord,
            "cost": cosT, "sint": sinT, "rmat": rm,
            "ones_col": np.ones((128, 1), np.float32),
            "ones_row": np.ones((1, 128), np.float32),
        })
    return in_maps


def _assemble(results):
    """Core c holds global output rows [256c, 256(c+1))."""
    out = np.concatenate([results[c]["out"] for c in range(NC)], axis=0)
    return out.reshape(1, S, H)


def kernel(positions, hidden_states, Wq, Wk, Wv, Wo):
    if "nc" not in _CACHE:
        _CACHE["nc"] = _build()
    nc = _CACHE["nc"]
    in_maps = _host_prep(positions, hidden_states, Wq, Wk, Wv, Wo)
    res = run_bass_kernel_spmd(nc, in_maps, list(range(NC)))
    return _assemble(res.results).astype(np.float32)
